# revision 3
# baseline (speedup 1.0000x reference)
"""Trainium2 Bass kernel for DiffusionConvolution (N=4096, F=16, K=3).

Reference computation:
    M = sum_k theta[k,0]*Wp[k] + theta[k,1]*WTp[k]        # [N, N]
    Y = X + M @ X

We never materialize M:
    Y = X + sum_t A_t @ (theta_t * X),   t in 0..5, A_t = Wp[k] / WTp[k]

Sharding: core c owns output rows [c*512, (c+1)*512). The TensorE contracts
over the partition dim, so each core gets the [4096, 512] column slice of
each A_t.T, packed host-side (layout prep) into 32 DMA-friendly slabs of
[128, 6*512]. The theta-scaled X ("x6") is replicated. Each matmul:
stationary = x6 chunk [128, 16], moving = A_t.T tile [128, 512] in
float32r (TF32-like, 1 cycle/row, rel err ~1e-4), accumulating all 192
(term, m-chunk) pairs into one [16, 512] PSUM bank. Output is Y.T per
core; host transposes + concatenates. No collectives.

Raw Bass (no TileContext): a linear 3-semaphore pipeline — sync engine
streams slabs (HWDGE FIFO, one dma sem), PE consumes with one wait per
slab group (the 4-byte fused-LDW matmul only supports a single sync
wait), DVE does the final X add. Per-core traffic 6*4096*512*4B = 48MB
-> ~135us at the ~358GB/s HBM-per-core limit; PE ~41us hides under DMA.
"""

import numpy as np

N = 4096
F = 16
K = 3
NCORES = 8
ROWS = N // NCORES            # 512 output rows per core
PART = 128                    # partition dim / contraction tile
MC = N // PART                # 32 contraction chunks
NTERMS = 2 * K                # 6 term matrices
WSLAB = NTERMS * ROWS         # 3072 free elems per m-chunk slab
NBUF = 3                      # W slab double/triple buffering

MOVING_DTYPE = "float32r"     # "float32" for exact (4x slower PE)


def _install_ntff_shim():
    """The image's antenv lacks axon_hooks; register the ctypes NTFF hook so
    run_bass_kernel_spmd(trace=True) works. Harmless no-op on failure."""
    import sys
    import types

    if "antenv.axon_hooks" in sys.modules:
        return
    try:
        from trn_agent_boot.trn_boot import _ntff_profile_via_ctypes

        hook = _ntff_profile_via_ctypes("/opt/axon/libaxon_pjrt.so")
        mod = types.ModuleType("antenv.axon_hooks")
        mod._hook = hook
        mod.get_axon_ntff_profile_hook = lambda: mod._hook
        mod.set_axon_ntff_profile_hook = lambda h: setattr(mod, "_hook", h)
        sys.modules["antenv.axon_hooks"] = mod
        try:
            import antenv

            antenv.axon_hooks = mod
        except Exception:
            pass
    except Exception:
        pass


_NC_CACHE = {}


def _build_bass():
    if "nc" in _NC_CACHE:
        return _NC_CACHE["nc"]
    import concourse.bass as bass  # noqa: F401
    import concourse.mybir as mybir

    f32 = mybir.dt.float32
    sb_dt = getattr(mybir.dt, MOVING_DTYPE)

    nc = bass.Bass(
        trn_type="TRN2",
        target_bir_lowering=False,
        debug=False,
        num_devices=NCORES,
    )
    wp = nc.dram_tensor("wpack", [MC, PART, WSLAB], f32, kind="ExternalInput")
    x6d = nc.dram_tensor("x6", [PART, MC * NTERMS * F], f32, kind="ExternalInput")
    xtd = nc.dram_tensor("xt", [F, ROWS], f32, kind="ExternalInput")
    outd = nc.dram_tensor("out", [F, ROWS], f32, kind="ExternalOutput")

    with (
        nc.semaphore("in_sem") as in_sem,
        nc.semaphore("slot_sem0") as slot_sem0,
        nc.semaphore("slot_sem1") as slot_sem1,
        nc.semaphore("slot_sem2") as slot_sem2,
        nc.semaphore("pe_sem") as pe_sem,
        nc.semaphore("dve_sem") as dve_sem,
        nc.semaphore("out_sem") as out_sem,
        nc.sbuf_tensor("x6s", [PART, MC * NTERMS * F], sb_dt) as x6s,
        nc.sbuf_tensor("xts", [F, ROWS], f32) as xts,
        nc.sbuf_tensor("wsl", [PART, NBUF * WSLAB], sb_dt) as wsl,
        nc.sbuf_tensor("osb", [F, ROWS], f32) as osb,
        nc.psum_tensor("acc", [F, ROWS], f32) as acc,
    ):
        slot_sems = [slot_sem0, slot_sem1, slot_sem2]
        assert NBUF == 3

        # Race-freedom: each slab slot has its own semaphore, and at most one
        # DMA per slot sem is ever in flight (the issue of slab mc+NBUF waits
        # on PE finishing group mc), so a wait for 16*(mc//NBUF+1) can only be
        # satisfied by slab mc's own 16 SDMA-engine increments.
        with nc.Block() as block:

            @block.sync
            def _(sync):
                sync.dma_start(x6s[:], x6d[:].bitcast(sb_dt)).then_inc(in_sem, 16)
                sync.dma_start(xts[:], xtd[:]).then_inc(in_sem, 16)
                for mc in range(MC):
                    if mc >= NBUF:
                        # WAR: don't overwrite slot until PE finished its group
                        sync.wait_ge(pe_sem, mc - NBUF + 1)
                    slot = (mc % NBUF) * WSLAB
                    sync.dma_start(
                        wsl[:, slot : slot + WSLAB], wp[mc].bitcast(sb_dt)
                    ).then_inc(slot_sems[mc % NBUF], 16)
                sync.wait_ge(dve_sem, 1)
                sync.dma_start(outd[:], osb[:]).then_inc(out_sem, 16)
                sync.wait_ge(out_sem, 16)

            @block.tensor
            def _(tensor):
                tensor.wait_ge(in_sem, 32)  # x6 (stationary for all groups)
                for mc in range(MC):
                    tensor.wait_ge(slot_sems[mc % NBUF], 16 * (mc // NBUF + 1))
                    slot = (mc % NBUF) * WSLAB
                    for t in range(NTERMS):
                        s = (mc * NTERMS + t) * F
                        mm = tensor.matmul(
                            acc[:],
                            lhsT=x6s[:, s : s + F],
                            rhs=wsl[:, slot + t * ROWS : slot + (t + 1) * ROWS],
                            start=(mc == 0 and t == 0),
                            stop=(mc == MC - 1 and t == NTERMS - 1),
                        )
                    mm.then_inc(pe_sem, 1)

            @block.vector
            def _(vector):
                vector.wait_ge(pe_sem, MC)
                vector.wait_ge(in_sem, 32)  # xt
                vector.tensor_add(osb[:], acc[:], xts[:]).then_inc(dve_sem, 1)

    _NC_CACHE["nc"] = nc
    return nc


def _pack_inputs(X, theta, Wp, WTp):
    X = np.ascontiguousarray(X, dtype=np.float32)
    theta = np.asarray(theta, dtype=np.float32)
    Wp = np.asarray(Wp, dtype=np.float32)
    WTp = np.asarray(WTp, dtype=np.float32)

    tf = theta.reshape(-1)  # t = 2k + j: [k0 Wp, k0 WTp, k1 Wp, ...]
    Xr = X.reshape(MC, PART, F)
    # x6[p, ((mc*NTERMS)+t)*F + f] = tf[t] * X[mc*PART + p, f]
    x6 = (
        (tf[None, None, :, None] * Xr.transpose(1, 0, 2)[:, :, None, :])
        .astype(np.float32)
        .reshape(PART, MC * NTERMS * F)
    )
    x6 = np.ascontiguousarray(x6)

    # pk[c, mc, p, t, nn] = A_t[c*ROWS + nn, mc*PART + p]
    pk = np.empty((NCORES, MC, PART, NTERMS, ROWS), dtype=np.float32)
    for k in range(K):
        for j, A in ((0, Wp[k]), (1, WTp[k])):
            t = 2 * k + j
            v = A.T.reshape(MC, PART, NCORES, ROWS)  # strided view, no copy
            pk[:, :, :, t, :] = v.transpose(2, 0, 1, 3)

    in_maps = []
    for c in range(NCORES):
        in_maps.append(
            {
                "wpack": pk[c].reshape(MC, PART, WSLAB),
                "x6": x6,
                "xt": np.ascontiguousarray(X[c * ROWS : (c + 1) * ROWS].T),
            }
        )
    return in_maps


def run(inputs, trace=False, trace_kwargs=None):
    """Returns (Y [N, F] float32, BassKernelResults)."""
    _install_ntff_shim()
    from concourse.bass_utils import run_bass_kernel_spmd

    nc = _build_bass()
    in_maps = _pack_inputs(**inputs)
    res = run_bass_kernel_spmd(
        nc,
        in_maps,
        core_ids=list(range(NCORES)),
        trace=trace,
        **(trace_kwargs or {}),
    )
    outs = [np.asarray(r["out"]) for r in res.results]
    Y = np.concatenate([o.T for o in outs], axis=0)
    return np.ascontiguousarray(Y, dtype=np.float32), res


def kernel(**inputs):
    Y, _ = run(inputs, trace=False)
    return Y


# revision 4
# speedup vs baseline: 1.0437x; 1.0437x over previous
"""Trainium2 Bass kernel for DiffusionConvolution (N=4096, F=16, K=3).

Reference computation:
    M = sum_k theta[k,0]*Wp[k] + theta[k,1]*WTp[k]        # [N, N]
    Y = X + M @ X

We never materialize M:
    Y = X + sum_t A_t @ (theta_t * X),   t in 0..5, A_t = Wp[k] / WTp[k]

Sharding: core c owns output rows [c*512, (c+1)*512). The TensorE contracts
over the partition dim, so each core gets the [4096, 512] column slice of
each A_t.T, packed host-side (layout prep) into 32 DMA-friendly slabs of
[128, 96 + 6*512] — the 96-wide head is that m-chunk's theta-scaled X
(stationary operands travel with their slab, so compute starts after the
first 1.6MB slab, with no separate X prefetch). Each matmul: stationary =
x6 head slice [128, 16], moving = A_t.T tile [128, 512] in float32r
(TF32-like, 1 cycle/row, rel err ~2e-4), accumulating all 192
(term, m-chunk) pairs into one [16, 512] PSUM bank. Output is Y.T per
core; host transposes + concatenates. No collectives.

Raw Bass (no TileContext): a linear pipeline on explicit semaphores.
The 4-byte fused-LDW matmul supports only ONE sync wait, and later DMA
completions on a shared semaphore can satisfy an earlier wait (16 SDMA
engines increment independently), so each of the NBUF slab slots gets
its own semaphore with at most one DMA in flight per sem — race-free by
construction. Per-core traffic ~50.6MB -> ~141us at the ~358GB/s
HBM-per-core limit; PE ~72us (HAM-throttled) hides under DMA.
"""

import numpy as np

N = 4096
F = 16
K = 3
NCORES = 8
ROWS = N // NCORES            # 512 output rows per core
PART = 128                    # partition dim / contraction tile
MC = N // PART                # 32 contraction chunks
NTERMS = 2 * K                # 6 term matrices
XHEAD = NTERMS * F            # 96 stationary elems per slab
WSLAB = XHEAD + NTERMS * ROWS  # 3168 free elems per m-chunk slab
NBUF = 8                      # W slab buffering depth

MOVING_DTYPE = "float32r"     # "float32" for exact (4x slower PE)


def _install_ntff_shim():
    """The image's antenv lacks axon_hooks; register the ctypes NTFF hook so
    run_bass_kernel_spmd(trace=True) works. Harmless no-op on failure."""
    import sys
    import types

    if "antenv.axon_hooks" in sys.modules:
        return
    try:
        from trn_agent_boot.trn_boot import _ntff_profile_via_ctypes

        hook = _ntff_profile_via_ctypes("/opt/axon/libaxon_pjrt.so")
        mod = types.ModuleType("antenv.axon_hooks")
        mod._hook = hook
        mod.get_axon_ntff_profile_hook = lambda: mod._hook
        mod.set_axon_ntff_profile_hook = lambda h: setattr(mod, "_hook", h)
        sys.modules["antenv.axon_hooks"] = mod
        try:
            import antenv

            antenv.axon_hooks = mod
        except Exception:
            pass
    except Exception:
        pass


_NC_CACHE = {}


def _build_bass():
    if "nc" in _NC_CACHE:
        return _NC_CACHE["nc"]
    import concourse.bass as bass  # noqa: F401
    import concourse.mybir as mybir

    f32 = mybir.dt.float32
    sb_dt = getattr(mybir.dt, MOVING_DTYPE)

    nc = bass.Bass(
        trn_type="TRN2",
        target_bir_lowering=False,
        debug=False,
        num_devices=NCORES,
    )
    wp = nc.dram_tensor("wpack", [MC, PART, WSLAB], f32, kind="ExternalInput")
    xtd = nc.dram_tensor("xt", [F, ROWS], f32, kind="ExternalInput")
    outd = nc.dram_tensor("out", [F, ROWS], f32, kind="ExternalOutput")

    with (
        nc.semaphore("in_sem") as in_sem,
        nc.semaphore("pe_sem") as pe_sem,
        nc.semaphore("dve_sem") as dve_sem,
        nc.semaphore("out_sem") as out_sem,
        nc.sbuf_tensor("xts", [F, ROWS], f32) as xts,
        nc.sbuf_tensor("wsl", [PART, NBUF * WSLAB], sb_dt) as wsl,
        nc.sbuf_tensor("osb", [F, ROWS], f32) as osb,
        nc.psum_tensor("acc", [F, ROWS], f32) as acc,
    ):
        import contextlib

        with contextlib.ExitStack() as st:
            slot_sems = [
                st.enter_context(nc.semaphore(f"slot_sem{i}")) for i in range(NBUF)
            ]

            # Race-freedom: each slab slot has its own semaphore and at most
            # one DMA per sem in flight (issuing slab mc+NBUF waits on PE
            # finishing group mc), so a wait for 16*(mc//NBUF+1) can only be
            # satisfied by slab mc's own 16 SDMA-engine increments.
            with nc.Block() as block:

                @block.sync
                def _(sync):
                    for mc in range(MC):
                        if mc >= NBUF:
                            # WAR: don't overwrite a slot PE hasn't consumed
                            sync.wait_ge(pe_sem, mc - NBUF + 1)
                        slot = (mc % NBUF) * WSLAB
                        sync.dma_start(
                            wsl[:, slot : slot + WSLAB], wp[mc].bitcast(sb_dt)
                        ).then_inc(slot_sems[mc % NBUF], 16)
                    sync.dma_start(xts[:], xtd[:]).then_inc(in_sem, 16)
                    sync.wait_ge(dve_sem, 1)
                    sync.dma_start(outd[:], osb[:]).then_inc(out_sem, 16)
                    sync.wait_ge(out_sem, 16)

                @block.tensor
                def _(tensor):
                    for mc in range(MC):
                        tensor.wait_ge(slot_sems[mc % NBUF], 16 * (mc // NBUF + 1))
                        slot = (mc % NBUF) * WSLAB
                        for t in range(NTERMS):
                            mm = tensor.matmul(
                                acc[:],
                                lhsT=wsl[:, slot + t * F : slot + (t + 1) * F],
                                rhs=wsl[
                                    :,
                                    slot + XHEAD + t * ROWS : slot
                                    + XHEAD
                                    + (t + 1) * ROWS,
                                ],
                                start=(mc == 0 and t == 0),
                                stop=(mc == MC - 1 and t == NTERMS - 1),
                            )
                        mm.then_inc(pe_sem, 1)

                @block.vector
                def _(vector):
                    vector.wait_ge(pe_sem, MC)
                    vector.wait_ge(in_sem, 16)  # xt
                    vector.tensor_add(osb[:], acc[:], xts[:]).then_inc(dve_sem, 1)

    _NC_CACHE["nc"] = nc
    return nc


def _pack_inputs(X, theta, Wp, WTp):
    X = np.ascontiguousarray(X, dtype=np.float32)
    theta = np.asarray(theta, dtype=np.float32)
    Wp = np.asarray(Wp, dtype=np.float32)
    WTp = np.asarray(WTp, dtype=np.float32)

    tf = theta.reshape(-1)  # t = 2k + j: [k0 Wp, k0 WTp, k1 Wp, ...]
    Xr = X.reshape(MC, PART, F)
    # x6[p, mc, t, f] = tf[t] * X[mc*PART + p, f]  (slab head, replicated)
    x6 = (
        tf[None, None, :, None] * Xr.transpose(1, 0, 2)[:, :, None, :]
    ).astype(np.float32)  # [PART, MC, NTERMS, F]

    # pk[c, mc, p, t, nn] = A_t[c*ROWS + nn, mc*PART + p]
    pk = np.empty((NCORES, MC, PART, WSLAB), dtype=np.float32)
    head = pk[:, :, :, :XHEAD].reshape(NCORES, MC, PART, NTERMS, F)
    head[:] = x6.transpose(1, 0, 2, 3)[None]  # same for every core
    body = pk[:, :, :, XHEAD:].reshape(NCORES, MC, PART, NTERMS, ROWS)
    for k in range(K):
        for j, A in ((0, Wp[k]), (1, WTp[k])):
            t = 2 * k + j
            v = A.T.reshape(MC, PART, NCORES, ROWS)  # strided view, no copy
            body[:, :, :, t, :] = v.transpose(2, 0, 1, 3)

    in_maps = []
    for c in range(NCORES):
        in_maps.append(
            {
                "wpack": pk[c],
                "xt": np.ascontiguousarray(X[c * ROWS : (c + 1) * ROWS].T),
            }
        )
    return in_maps


def run(inputs, trace=False, trace_kwargs=None):
    """Returns (Y [N, F] float32, BassKernelResults)."""
    _install_ntff_shim()
    from concourse.bass_utils import run_bass_kernel_spmd

    nc = _build_bass()
    in_maps = _pack_inputs(**inputs)
    res = run_bass_kernel_spmd(
        nc,
        in_maps,
        core_ids=list(range(NCORES)),
        trace=trace,
        **(trace_kwargs or {}),
    )
    outs = [np.asarray(r["out"]) for r in res.results]
    Y = np.concatenate([o.T for o in outs], axis=0)
    return np.ascontiguousarray(Y, dtype=np.float32), res


def kernel(**inputs):
    Y, _ = run(inputs, trace=False)
    return Y


# revision 7
# speedup vs baseline: 1.7377x; 1.6649x over previous
"""Trainium2 Bass kernel for DiffusionConvolution (N=4096, F=16, K=3).

Reference computation:
    M = sum_k theta[k,0]*Wp[k] + theta[k,1]*WTp[k]        # [N, N]
    Y = X + M @ X

We never materialize M:
    Y = X + sum_t A_t @ (theta_t * X)   over the 2K term matrices.

Wp[0] and WTp[0] are identity matrices by construction (k=0 diffusion
power), so their terms reduce to (theta[0,0]+theta[0,1])*X and are folded
into the final X add — verified exactly at runtime with a fallback to the
general path. That cuts streamed W data by 1/3.

Sharding: core c owns output rows [c*512, (c+1)*512). The TensorE
contracts over the partition dim, so each core gets the [4096, 512]
column slice of each remaining A_t.T, packed host-side into 32
DMA-friendly slabs of [128, nt*16 + nt*512] whose head carries that
m-chunk's theta-scaled X (stationary operands travel with their slab).
Each matmul: stationary = head slice [128, 16], moving = A_t.T tile
[128, 512] in float32r (TF32-like, 1 cycle/row, rel err ~2e-4),
accumulating all nt*32 pairs into one [16, 512] PSUM bank. Output is
Y.T per core; host transposes + concatenates. No collectives.

Raw Bass (no TileContext): a linear pipeline on explicit semaphores.
The 4-byte fused-LDW matmul supports only ONE sync wait, and later DMA
completions on a shared semaphore can satisfy an earlier wait (16 SDMA
engines increment independently), so each of the NBUF slab slots gets
its own semaphore with at most one DMA in flight per sem — race-free by
construction.
"""

import numpy as np

N = 4096
F = 16
K = 3
NCORES = 8
ROWS = N // NCORES            # 512 output rows per core
PART = 128                    # partition dim / contraction tile
MC = N // PART                # 32 contraction chunks
NBUF = 12                     # W slab buffering depth

MOVING_DTYPE = "float32r"     # "float32" for exact (4x slower PE)


def _install_ntff_shim():
    """The image's antenv lacks axon_hooks; register the ctypes NTFF hook so
    run_bass_kernel_spmd(trace=True) works. Harmless no-op on failure."""
    import sys
    import types

    if "antenv.axon_hooks" in sys.modules:
        return
    try:
        from trn_agent_boot.trn_boot import _ntff_profile_via_ctypes

        hook = _ntff_profile_via_ctypes("/opt/axon/libaxon_pjrt.so")
        mod = types.ModuleType("antenv.axon_hooks")
        mod._hook = hook
        mod.get_axon_ntff_profile_hook = lambda: mod._hook
        mod.set_axon_ntff_profile_hook = lambda h: setattr(mod, "_hook", h)
        sys.modules["antenv.axon_hooks"] = mod
        try:
            import antenv

            antenv.axon_hooks = mod
        except Exception:
            pass
    except Exception:
        pass


_NC_CACHE = {}


def _build_bass(nt):
    """Bass graph for nt term matrices. Slab = [PART, nt*(F + ROWS)]."""
    if nt in _NC_CACHE:
        return _NC_CACHE[nt]
    import contextlib

    import concourse.bass as bass  # noqa: F401
    import concourse.mybir as mybir

    f32 = mybir.dt.float32
    sb_dt = getattr(mybir.dt, MOVING_DTYPE)
    xhead = nt * F
    wslab = xhead + nt * ROWS

    nc = bass.Bass(
        trn_type="TRN2",
        target_bir_lowering=False,
        debug=False,
        num_devices=NCORES,
    )
    wp = nc.dram_tensor("wpack", [MC, PART, wslab], f32, kind="ExternalInput")
    xtd = nc.dram_tensor("xt", [F, ROWS], f32, kind="ExternalInput")
    outd = nc.dram_tensor("out", [F, ROWS], f32, kind="ExternalOutput")

    with (
        nc.semaphore("in_sem") as in_sem,
        nc.semaphore("pe_sem") as pe_sem,
        nc.semaphore("dve_sem") as dve_sem,
        nc.semaphore("out_sem") as out_sem,
        nc.sbuf_tensor("xts", [F, ROWS], f32) as xts,
        nc.sbuf_tensor("wsl", [PART, NBUF * wslab], sb_dt) as wsl,
        nc.sbuf_tensor("osb", [F, ROWS], f32) as osb,
        nc.psum_tensor("acc", [F, ROWS], f32) as acc,
        contextlib.ExitStack() as st,
    ):
        slot_sems = [
            st.enter_context(nc.semaphore(f"slot_sem{i}")) for i in range(NBUF)
        ]

        with nc.Block() as block:

            @block.sync
            def _(sync):
                for mc in range(MC):
                    if mc >= NBUF:
                        # WAR: don't overwrite a slot PE hasn't consumed
                        sync.wait_ge(pe_sem, mc - NBUF + 1)
                    slot = (mc % NBUF) * wslab
                    sync.dma_start(
                        wsl[:, slot : slot + wslab], wp[mc].bitcast(sb_dt)
                    ).then_inc(slot_sems[mc % NBUF], 16)
                sync.dma_start(xts[:], xtd[:]).then_inc(in_sem, 16)
                sync.wait_ge(dve_sem, 1)
                sync.dma_start(outd[:], osb[:]).then_inc(out_sem, 16)
                sync.wait_ge(out_sem, 16)

            @block.tensor
            def _(tensor):
                for mc in range(MC):
                    tensor.wait_ge(slot_sems[mc % NBUF], 16 * (mc // NBUF + 1))
                    slot = (mc % NBUF) * wslab
                    for t in range(nt):
                        mm = tensor.matmul(
                            acc[:],
                            lhsT=wsl[:, slot + t * F : slot + (t + 1) * F],
                            rhs=wsl[
                                :,
                                slot + xhead + t * ROWS : slot
                                + xhead
                                + (t + 1) * ROWS,
                            ],
                            start=(mc == 0 and t == 0),
                            stop=(mc == MC - 1 and t == nt - 1),
                        )
                    mm.then_inc(pe_sem, 1)

            @block.vector
            def _(vector):
                vector.wait_ge(pe_sem, MC)
                vector.wait_ge(in_sem, 16)  # xt
                vector.tensor_add(osb[:], acc[:], xts[:]).then_inc(dve_sem, 1)

    _NC_CACHE[nt] = nc
    return nc


def _is_identity(A):
    """Exact check: A == eye(N), without materializing eye."""
    if np.count_nonzero(A) != N:
        return False
    return bool((np.diagonal(A) == 1.0).all())


def _pack_inputs(X, theta, Wp, WTp):
    X = np.ascontiguousarray(X, dtype=np.float32)
    theta = np.asarray(theta, dtype=np.float32)
    Wp = np.asarray(Wp, dtype=np.float32)
    WTp = np.asarray(WTp, dtype=np.float32)

    # Identity terms contribute theta*X directly; fold into the X add.
    terms = []       # (scale, matrix) for non-identity terms
    xscale = 1.0     # Y = X + ... -> the "1"
    for k in range(K):
        for j, A in ((0, Wp[k]), (1, WTp[k])):
            th = float(theta[k, j])
            if k == 0 and _is_identity(A):
                xscale += th
            else:
                terms.append((th, A))
    nt = len(terms)

    xhead = nt * F
    wslab = xhead + nt * ROWS
    Xr = X.reshape(MC, PART, F)

    # pk[c, mc, p, :xhead]  = per-term theta-scaled X chunk (same all cores)
    # pk[c, mc, p, xhead:]  reshaped [nt, ROWS]: A_t[c*ROWS + nn, mc*PART + p]
    pk = np.empty((NCORES, MC, PART, wslab), dtype=np.float32)
    head = pk[:, :, :, :xhead].reshape(NCORES, MC, PART, nt, F)
    body = pk[:, :, :, xhead:].reshape(NCORES, MC, PART, nt, ROWS)
    for t, (th, A) in enumerate(terms):
        v = A.T.reshape(MC, PART, NCORES, ROWS)  # strided view, no copy
        body[:, :, :, t, :] = v.transpose(2, 0, 1, 3)
    # head[c, mc, p, t, f] = th_t * X[mc*PART + p, f]
    hx = np.stack([th * Xr for th, _ in terms], axis=2)  # [MC, PART, nt, F]
    head[:] = hx[None]

    in_maps = []
    for c in range(NCORES):
        in_maps.append(
            {
                "wpack": pk[c],
                "xt": np.ascontiguousarray(
                    (xscale * X[c * ROWS : (c + 1) * ROWS]).T
                ),
            }
        )
    return in_maps, nt


def run(inputs, trace=False, trace_kwargs=None):
    """Returns (Y [N, F] float32, BassKernelResults)."""
    _install_ntff_shim()
    from concourse.bass_utils import run_bass_kernel_spmd

    in_maps, nt = _pack_inputs(**inputs)
    nc = _build_bass(nt)
    res = run_bass_kernel_spmd(
        nc,
        in_maps,
        core_ids=list(range(NCORES)),
        trace=trace,
        **(trace_kwargs or {}),
    )
    outs = [np.asarray(r["out"]) for r in res.results]
    Y = np.concatenate([o.T for o in outs], axis=0)
    return np.ascontiguousarray(Y, dtype=np.float32), res


def kernel(**inputs):
    Y, _ = run(inputs, trace=False)
    return Y
